# revision 12
# baseline (speedup 1.0000x reference)
"""Trainium2 Bass kernel for nn_Attention_15556371546220 (Enformer-style
relative-position attention, B=1 L=4096 C=768 H=4 DK=64 DV=192 POSF=64).

Sharding: 8 cores = 4 heads x 2 query-blocks of 2048. Each core computes its
head's K/V over the full sequence, Q over its query block, full attention with
the relative-shift positional term, and a partial output projection
(row-parallel over the head's 192 value dims). Host sums the 4 head partials
per query block and adds the output bias.

v3 structure:
- exp factorization: exp(content + shifted_pos) = exp(content) * exp(U)|skew.
  The U eviction PSUM->SBUF becomes an Exp activation (same scalar cost as a
  copy), the DRAM scratch holds exp(U) in bf16, and the old fp32-PSUM add on
  DVE becomes an SBUF 16-bit multiply (2x DVE rate). bf16's exponent range
  makes this safe without any max-subtraction (logits stay well under 88).
- softmax denominator via a ones-column appended to V: o_psum[:, 192] = z.
- o computed query-major; z reciprocal from column 192; out scaled on evict.
- one batched skew-read DMA per query tile ([128,4096], row stride 4223 over
  a pitch-4224 DRAM scratch realizes the relative shift).
- software-pipelined phase B (U emit at t+2, skew read at t+1, attention at t,
  o-acc + projection at t-1) to keep the tensor engine dense.
- fp16 partial outputs (host accumulates in fp32).
"""
import sys
if "/opt/trn_rl_repo" not in sys.path:
    sys.path.insert(0, "/opt/trn_rl_repo")

import numpy as np
import ml_dtypes

import concourse.bass as bass
import concourse.bacc as bacc
import concourse.mybir as mybir
import concourse.tile as tile
from concourse.bass_utils import run_bass_kernel_spmd

F32 = mybir.dt.float32
BF16 = mybir.dt.bfloat16
FP16 = mybir.dt.float16
AX = mybir.AxisListType
ALU = mybir.AluOpType
ACT = mybir.ActivationFunctionType

B, L, C = 1, 4096, 768
H, DK, DV = 4, 64, 192
POSF = 64
NQ = 2048          # queries per core (one of two blocks)
NT = 16            # query tiles of 128 per core
UW = 4223          # U window width per query tile
UP = 4224          # U row pitch in DRAM scratch
PKW = 6144         # per-core pos-key window (covers all 16 tiles)

_nc_cache = {}


def _build_nc():
    nc = bacc.Bacc()

    xt_in = nc.declare_dram_parameter("xt", (C, L), FP16, isOutput=False)
    xq_in = nc.declare_dram_parameter("xq", (C, NQ), FP16, isOutput=False)
    wq_in = nc.declare_dram_parameter("wq", (C, DK), FP16, isOutput=False)
    wk_in = nc.declare_dram_parameter("wk", (C, DK), FP16, isOutput=False)
    wv_in = nc.declare_dram_parameter("wv", (C, DV), FP16, isOutput=False)
    wpos_in = nc.declare_dram_parameter("wpos", (POSF, DK), FP16, isOutput=False)
    post_in = nc.declare_dram_parameter("post", (POSF, PKW), FP16, isOutput=False)
    wout_in = nc.declare_dram_parameter("wout", (DV, C), BF16, isOutput=False)
    rcb_in = nc.declare_dram_parameter("rcb", (DK, 1), F32, isOutput=False)
    rpb_in = nc.declare_dram_parameter("rpb", (DK, 1), F32, isOutput=False)
    ident_in = nc.declare_dram_parameter("ident", (128, 128), BF16, isOutput=False)
    out_dram = nc.declare_dram_parameter("out", (NQ, C), FP16, isOutput=True)

    with tile.TileContext(nc) as tc:
        with (
            tc.tile_pool(name="const", bufs=1) as cpool,
            tc.tile_pool(name="res", bufs=1) as rpool,
            tc.tile_pool(name="udram", bufs=3, space="DRAM") as dpool,
        ):
            # ---------- inputs: compute-critical DMAs first ----------
            post_sb = cpool.tile([POSF, PKW], FP16)
            nc.sync.dma_start(post_sb[:], post_in[:])
            xq_sb = rpool.tile([128, 6, NQ], FP16)     # x^T, query block
            for cc in range(6):
                nc.sync.dma_start(xq_sb[:, cc, :],
                                  xq_in[cc * 128:(cc + 1) * 128, :])
            xt_sb = rpool.tile([128, 6, L], FP16)      # x^T, full seq
            for cc in range(6):
                nc.sync.dma_start(xt_sb[:, cc, :],
                                  xt_in[cc * 128:(cc + 1) * 128, :])
            # constants on gpsimd queue (order matters: wpos/wq feed phase A)
            wpos_sb = cpool.tile([POSF, DK], FP16)
            nc.gpsimd.dma_start(wpos_sb[:], wpos_in[:])
            wq_sb = cpool.tile([128, 6, DK], FP16)
            nc.gpsimd.dma_start(wq_sb[:], wq_in.rearrange("(cc p) d -> p cc d", p=128))
            rcb_sb = cpool.tile([DK, 1], F32)
            nc.gpsimd.dma_start(rcb_sb[:], rcb_in[:])
            rpb_sb = cpool.tile([DK, 1], F32)
            nc.gpsimd.dma_start(rpb_sb[:], rpb_in[:])
            wk_sb = cpool.tile([128, 6, DK], FP16)
            nc.gpsimd.dma_start(wk_sb[:], wk_in.rearrange("(cc p) d -> p cc d", p=128))
            wv_sb = cpool.tile([128, 6, DV], FP16)
            nc.gpsimd.dma_start(wv_sb[:], wv_in.rearrange("(cc p) d -> p cc d", p=128))
            ident_sb = cpool.tile([128, 128], BF16)
            nc.gpsimd.dma_start(ident_sb[:], ident_in[:])
            wout1_sb = cpool.tile([128, C], BF16)
            nc.gpsimd.dma_start(wout1_sb[:], wout_in[0:128, :])
            wout2_sb = cpool.tile([64, C], BF16)
            nc.gpsimd.dma_start(wout2_sb[:], wout_in[128:192, :])

            # ---------- residents ----------
            kt_sb = rpool.tile([DK, L], FP16)          # K^T
            vp_sb = rpool.tile([128, 32, DV + 1], BF16)  # V per key blk + ones
            pkt_sb = rpool.tile([DK, PKW], FP16)       # pos_k^T window
            qct_sb = rpool.tile([DK, NQ], FP16)        # (Q/8 + rcb)^T
            qpt_sb = rpool.tile([DK, NQ], FP16)        # (Q/8 + rpb)^T

            nc.vector.memset(vp_sb[:, :, DV:DV + 1], 1.0)

            with (
                tc.tile_pool(name="ub", bufs=2) as upool,
                tc.tile_pool(name="usk", bufs=2) as uskpool,
                tc.tile_pool(name="ec", bufs=4) as ecpool,
                tc.tile_pool(name="at", bufs=2) as apool,
                tc.tile_pool(name="att", bufs=2) as atpool,
                tc.tile_pool(name="zz", bufs=4) as zpool,
                tc.tile_pool(name="ot", bufs=2) as opool,
                tc.tile_pool(name="fin", bufs=2) as fpool,
                tc.tile_pool(name="psU", bufs=2, space="PSUM") as psU,
                tc.tile_pool(name="psC", bufs=3, space="PSUM") as psC,
                tc.tile_pool(name="psT", bufs=1, space="PSUM") as psT,
                tc.tile_pool(name="psO", bufs=1, space="PSUM") as psO,
                tc.tile_pool(name="psP", bufs=1, space="PSUM") as psP,
            ):
                # ---------- phase A: projections ----------
                # pos_k^T window (only needs post + wpos)
                for mc in range(12):
                    pk_ps = psU.tile([128, 512], F32, tag="psu")
                    nc.tensor.matmul(pk_ps[0:DK, :], wpos_sb[:],
                                     post_sb[:, mc * 512:(mc + 1) * 512],
                                     start=True, stop=True)
                    nc.vector.tensor_copy(
                        pkt_sb[:, mc * 512:(mc + 1) * 512], pk_ps[0:DK, :])
                # Q (+ biases, /8) -> qct, qpt   (only needs xq + wq)
                for qc in range(4):
                    q_ps = psC.tile([128, 512], F32, tag="psc")
                    for cc in range(6):
                        nc.tensor.matmul(q_ps[0:DK, :], wq_sb[:, cc, :],
                                         xq_sb[:, cc, qc * 512:(qc + 1) * 512],
                                         start=(cc == 0), stop=(cc == 5))
                    nc.scalar.activation(qct_sb[:, qc * 512:(qc + 1) * 512],
                                         q_ps[0:DK, :], ACT.Identity,
                                         bias=rcb_sb[:], scale=0.125)
                    nc.scalar.activation(qpt_sb[:, qc * 512:(qc + 1) * 512],
                                         q_ps[0:DK, :], ACT.Identity,
                                         bias=rpb_sb[:], scale=0.125)

                def emit_u(qt):
                    """u_sb = exp(U);  U[p, m] = y_p . pkt[ws + m], w 4223."""
                    ws = 1920 - 128 * qt
                    u_sb = upool.tile([128, UP], BF16, tag="u")
                    for uc in range(9):
                        w = 512 if uc < 8 else UW - 8 * 512
                        u_ps = psU.tile([128, 512], F32, tag="psu")
                        nc.tensor.matmul(
                            u_ps[:, 0:w],
                            qpt_sb[:, qt * 128:(qt + 1) * 128],
                            pkt_sb[:, ws + uc * 512: ws + uc * 512 + w],
                            start=True, stop=True)
                        nc.scalar.activation(
                            u_sb[:, uc * 512: uc * 512 + w], u_ps[:, 0:w],
                            ACT.Exp)
                    u_dr = dpool.tile([128, UP], BF16, tag="udr")
                    nc.sync.dma_start(u_dr[:], u_sb[:])
                    return u_dr

                def emit_skew(u_dr):
                    """eusk[p, j] = expU[p, j + 127 - p] via strided read."""
                    usk = uskpool.tile([128, L], BF16, tag="usk")
                    skew = bass.AP(u_dr.tensor, u_dr[:].offset + 127,
                                   [[UW, 128], [1, L]])
                    nc.gpsimd.dma_start(usk[:], skew)
                    return usk

                def emit_content(qt, usk, chunks):
                    """content matmul + exp + multiply-by-exp(shifted pos)."""
                    attn = state[qt]["attn"]
                    for ch in chunks:
                        c_ps = psC.tile([128, 512], F32, tag="psc")
                        nc.tensor.matmul(
                            c_ps[:], qct_sb[:, qt * 128:(qt + 1) * 128],
                            kt_sb[:, ch * 512:(ch + 1) * 512],
                            start=True, stop=True)
                        ec = ecpool.tile([128, 512], BF16, tag="ec")
                        nc.scalar.activation(ec[:], c_ps[:], ACT.Exp)
                        nc.vector.tensor_tensor(
                            attn[:, ch * 512:(ch + 1) * 512], ec[:],
                            usk[:, ch * 512:(ch + 1) * 512], ALU.mult)

                def emit_transpose(qt, half):
                    """attnT[j, kb, q] = attn[q, 128*kb + j] via DMA XBAR.

                    Issued per 2048-key half so the transpose overlaps the
                    exp/multiply of the remaining chunks.
                    """
                    attn = state[qt]["attn"]
                    if half == 0:
                        attnT = atpool.tile([128, 32, 128], BF16, tag="attnT")
                        state[qt]["attnT"] = attnT
                    attnT = state[qt]["attnT"]
                    nc.sync.dma_start_transpose(
                        attnT[:, half * 16:(half + 1) * 16, :],
                        attn[:, half * 2048:(half + 1) * 2048])

                def emit_oacc_proj(qt):
                    attnT = state[qt]["attnT"]
                    o_ps = psO.tile([128, DV + 1], F32, tag="po")
                    for kb in range(32):
                        nc.tensor.matmul(o_ps[:], attnT[:, kb, :],
                                         vp_sb[:, kb, :],
                                         start=(kb == 0), stop=(kb == 31))
                    rz = zpool.tile([128, 1], F32, tag="rz")
                    nc.vector.reciprocal(rz[:], o_ps[:, DV:DV + 1])
                    o_sb = opool.tile([128, DV], BF16, tag="o")
                    nc.vector.tensor_copy(o_sb[:], o_ps[:, 0:DV])
                    # transpose o -> oT (128+64 partitions)
                    t_ps = psT.tile([128, 512], BF16, tag="pst")
                    nc.tensor.transpose(t_ps[:, 0:128], o_sb[:, 0:128],
                                        ident_sb[:])
                    nc.tensor.transpose(t_ps[0:64, 128:256], o_sb[:, 128:192],
                                        ident_sb[:])
                    oT1 = opool.tile([128, 128], BF16, tag="oT1")
                    nc.vector.tensor_copy(oT1[:], t_ps[:, 0:128])
                    oT2 = opool.tile([64, 128], BF16, tag="oT2")
                    nc.vector.tensor_copy(oT2[:], t_ps[0:64, 128:256])
                    fin = fpool.tile([128, C], FP16, tag="fin")
                    for n0 in (0, 384):
                        p_ps = psP.tile([128, 384], F32, tag="pp")
                        nc.tensor.matmul(p_ps[:], oT1[:],
                                         wout1_sb[:, n0:n0 + 384],
                                         start=True, stop=False)
                        nc.tensor.matmul(p_ps[:], oT2[:],
                                         wout2_sb[:, n0:n0 + 384],
                                         start=False, stop=True)
                        nc.vector.tensor_scalar_mul(fin[:, n0:n0 + 384],
                                                    p_ps[:], rz[:])
                    nc.gpsimd.dma_start(
                        out_dram[qt * 128:(qt + 1) * 128, :], fin[:])

                state = [dict() for _ in range(NT)]
                # prime U(0) early: it only needs qpt + pkt (fills the
                # tensor-engine gap while xt still streams in for K)
                state[0]["udr"] = emit_u(0)

                # K^T (needs xt)
                for kc in range(8):
                    kt_ps = psC.tile([128, 512], F32, tag="psc")
                    for cc in range(6):
                        nc.tensor.matmul(kt_ps[0:DK, :], wk_sb[:, cc, :],
                                         xt_sb[:, cc, kc * 512:(kc + 1) * 512],
                                         start=(cc == 0), stop=(cc == 5))
                    if kc % 2 == 0:
                        nc.vector.tensor_copy(
                            kt_sb[:, kc * 512:(kc + 1) * 512], kt_ps[0:DK, :])
                    else:
                        nc.scalar.copy(
                            kt_sb[:, kc * 512:(kc + 1) * 512], kt_ps[0:DK, :])

                state[1]["udr"] = emit_u(1)
                state[0]["usk"] = emit_skew(state[0]["udr"])

                # V (key-major, ones column preset)
                for jb in range(32):
                    v_ps = psC.tile([128, 512], F32, tag="psc")
                    for cc in range(6):
                        nc.tensor.matmul(
                            v_ps[:, 0:DV],
                            xt_sb[:, cc, jb * 128:(jb + 1) * 128],
                            wv_sb[:, cc, :], start=(cc == 0), stop=(cc == 5))
                    nc.vector.tensor_copy(vp_sb[:, jb, 0:DV], v_ps[:, 0:DV])

                # ---------- phase B steady loop ----------
                for t in range(NT):
                    attn_t = apool.tile([128, L], BF16, tag="attn")
                    state[t]["attn"] = attn_t
                    emit_content(t, state[t]["usk"], range(0, 4))
                    emit_transpose(t, 0)
                    if t + 2 < NT:
                        state[t + 2]["udr"] = emit_u(t + 2)
                    if t + 1 < NT:
                        state[t + 1]["usk"] = emit_skew(state[t + 1]["udr"])
                    emit_content(t, state[t]["usk"], range(4, 7))
                    if t > 0:
                        emit_oacc_proj(t - 1)
                    emit_content(t, state[t]["usk"], range(7, 8))
                    emit_transpose(t, 1)
                emit_oacc_proj(NT - 1)

    nc.finalize()
    return nc


def _positions_T():
    feat = POSF // 2
    pow_rate = np.exp(np.log(L + 1) / feat).astype(np.float64)
    pos = np.arange(-L + 1, L, dtype=np.float64)                 # (8191,)
    cw = pow_rate ** np.arange(1, feat + 1, dtype=np.float64) - 1.0
    emb = (cw[None, :] > np.abs(pos)[:, None]).astype(np.float32)
    signed = np.sign(pos)[:, None].astype(np.float32) * emb
    p = np.concatenate([emb, signed], axis=-1)                   # (8191, 64)
    pt = np.zeros((POSF, 2 * L), np.float32)
    pt[:, :2 * L - 1] = p.T
    return pt


def kernel(x, Wq, Wk, Wv, Wpos, Wout, bout, rel_content_bias, rel_pos_bias):
    bf = ml_dtypes.bfloat16
    f16 = np.float16
    if "nc" not in _nc_cache:
        _nc_cache["nc"] = _build_nc()
    nc = _nc_cache["nc"]

    xt = np.ascontiguousarray(x[0].T).astype(f16)                 # (C, L)
    posT = _positions_T()                                        # (64, 8192)
    ident = np.eye(128, dtype=bf)

    in_maps = []
    for c in range(8):
        h, b = c // 2, c % 2
        w0 = 3968 - 2048 * b
        in_maps.append({
            "xt": xt,
            "xq": np.ascontiguousarray(x[0, b * NQ:(b + 1) * NQ].T).astype(f16),
            "wq": Wq[:, h * DK:(h + 1) * DK].astype(f16),
            "wk": Wk[:, h * DK:(h + 1) * DK].astype(f16),
            "wv": Wv[:, h * DV:(h + 1) * DV].astype(f16),
            "wpos": Wpos[:, h * DK:(h + 1) * DK].astype(f16),
            "post": np.ascontiguousarray(
                posT[:, w0 - 1920: w0 - 1920 + PKW]).astype(f16),
            "wout": Wout[h * DV:(h + 1) * DV, :].astype(bf),
            "rcb": np.ascontiguousarray(
                rel_content_bias[0, h, 0][:, None]).astype(np.float32),
            "rpb": np.ascontiguousarray(
                rel_pos_bias[0, h, 0][:, None]).astype(np.float32),
            "ident": ident,
        })

    res = run_bass_kernel_spmd(nc, in_maps, core_ids=list(range(8)))
    globals()["last_results"] = res
    parts = [r["out"] for r in res.results]

    out = np.zeros((L, C), np.float32)
    for b in range(2):
        acc = np.zeros((NQ, C), np.float32)
        for h in range(4):
            acc += parts[h * 2 + b].astype(np.float32)
        out[b * NQ:(b + 1) * NQ] = acc
    out += bout[None, :].astype(np.float32)
    return out.reshape(1, L, C)


# revision 14
# speedup vs baseline: 1.0271x; 1.0271x over previous
"""Trainium2 Bass kernel for nn_Attention_15556371546220 (Enformer-style
relative-position attention, B=1 L=4096 C=768 H=4 DK=64 DV=192 POSF=64).

Sharding: 8 cores = 4 heads x 2 query-blocks of 2048. Each core computes its
head's K/V over the full sequence, Q over its query block, full attention with
the relative-shift positional term, and a partial output projection
(row-parallel over the head's 192 value dims). Host sums the 4 head partials
per query block and adds the output bias.

v3 structure:
- exp factorization: exp(content + shifted_pos) = exp(content) * exp(U)|skew.
  The U eviction PSUM->SBUF becomes an Exp activation (same scalar cost as a
  copy), the DRAM scratch holds exp(U) in bf16, and the old fp32-PSUM add on
  DVE becomes an SBUF 16-bit multiply (2x DVE rate). bf16's exponent range
  makes this safe without any max-subtraction (logits stay well under 88).
- softmax denominator via a ones-column appended to V: o_psum[:, 192] = z.
- o computed query-major; z reciprocal from column 192; out scaled on evict.
- one batched skew-read DMA per query tile ([128,4096], row stride 4223 over
  a pitch-4224 DRAM scratch realizes the relative shift).
- software-pipelined phase B (U emit at t+2, skew read at t+1, attention at t,
  o-acc + projection at t-1) to keep the tensor engine dense.
- fp16 partial outputs (host accumulates in fp32).
"""
import sys
if "/opt/trn_rl_repo" not in sys.path:
    sys.path.insert(0, "/opt/trn_rl_repo")

import numpy as np
import ml_dtypes

import concourse.bass as bass
import concourse.bacc as bacc
import concourse.mybir as mybir
import concourse.tile as tile
from concourse.bass_utils import run_bass_kernel_spmd

F32 = mybir.dt.float32
BF16 = mybir.dt.bfloat16
FP16 = mybir.dt.float16
AX = mybir.AxisListType
ALU = mybir.AluOpType
ACT = mybir.ActivationFunctionType

B, L, C = 1, 4096, 768
H, DK, DV = 4, 64, 192
POSF = 64
NQ = 2048          # queries per core (one of two blocks)
NT = 16            # query tiles of 128 per core
UW = 4223          # U window width per query tile
UP = 4224          # U row pitch in DRAM scratch
PKW = 6144         # per-core pos-key window (covers all 16 tiles)

_nc_cache = {}


def _build_nc():
    nc = bacc.Bacc()

    xt_in = nc.declare_dram_parameter("xt", (C, L), FP16, isOutput=False)
    xq_in = nc.declare_dram_parameter("xq", (C, NQ), FP16, isOutput=False)
    wq_in = nc.declare_dram_parameter("wq", (C, DK), FP16, isOutput=False)
    wk_in = nc.declare_dram_parameter("wk", (C, DK), FP16, isOutput=False)
    wv_in = nc.declare_dram_parameter("wv", (C, DV), FP16, isOutput=False)
    wpos_in = nc.declare_dram_parameter("wpos", (POSF, DK), FP16, isOutput=False)
    post_in = nc.declare_dram_parameter("post", (POSF, PKW), FP16, isOutput=False)
    wout_in = nc.declare_dram_parameter("wout", (DV, C), BF16, isOutput=False)
    rcb_in = nc.declare_dram_parameter("rcb", (DK, 1), F32, isOutput=False)
    rpb_in = nc.declare_dram_parameter("rpb", (DK, 1), F32, isOutput=False)
    ident_in = nc.declare_dram_parameter("ident", (128, 128), BF16, isOutput=False)
    out_dram = nc.declare_dram_parameter("out", (NQ, C), FP16, isOutput=True)

    with tile.TileContext(nc) as tc:
        with (
            tc.tile_pool(name="const", bufs=1) as cpool,
            tc.tile_pool(name="res", bufs=1) as rpool,
            tc.tile_pool(name="udram", bufs=3, space="DRAM") as dpool,
        ):
            # ---------- inputs: compute-critical DMAs first ----------
            post_sb = cpool.tile([POSF, PKW], FP16)
            nc.sync.dma_start(post_sb[:], post_in[:])
            xq_sb = rpool.tile([128, 6, NQ], FP16)     # x^T, query block
            for cc in range(6):
                nc.sync.dma_start(xq_sb[:, cc, :],
                                  xq_in[cc * 128:(cc + 1) * 128, :])
            xt_sb = rpool.tile([128, 6, L], FP16)      # x^T, full seq
            for cc in range(6):
                nc.sync.dma_start(xt_sb[:, cc, :],
                                  xt_in[cc * 128:(cc + 1) * 128, :])
            # constants on gpsimd queue (order matters: wpos/wq feed phase A)
            wpos_sb = cpool.tile([POSF, DK], FP16)
            nc.gpsimd.dma_start(wpos_sb[:], wpos_in[:])
            wq_sb = cpool.tile([128, 6, DK], FP16)
            nc.gpsimd.dma_start(wq_sb[:], wq_in.rearrange("(cc p) d -> p cc d", p=128))
            rcb_sb = cpool.tile([DK, 1], F32)
            nc.gpsimd.dma_start(rcb_sb[:], rcb_in[:])
            rpb_sb = cpool.tile([DK, 1], F32)
            nc.gpsimd.dma_start(rpb_sb[:], rpb_in[:])
            wk_sb = cpool.tile([128, 6, DK], FP16)
            nc.gpsimd.dma_start(wk_sb[:], wk_in.rearrange("(cc p) d -> p cc d", p=128))
            wv_sb = cpool.tile([128, 6, DV], FP16)
            nc.gpsimd.dma_start(wv_sb[:], wv_in.rearrange("(cc p) d -> p cc d", p=128))
            ident_sb = cpool.tile([128, 128], BF16)
            nc.gpsimd.dma_start(ident_sb[:], ident_in[:])
            wout1_sb = cpool.tile([128, C], BF16)
            nc.gpsimd.dma_start(wout1_sb[:], wout_in[0:128, :])
            wout2_sb = cpool.tile([64, C], BF16)
            nc.gpsimd.dma_start(wout2_sb[:], wout_in[128:192, :])

            # ---------- residents ----------
            kt_sb = rpool.tile([DK, L], FP16)          # K^T
            vp_sb = rpool.tile([128, 32, DV + 1], BF16)  # V per key blk + ones
            pkt_sb = rpool.tile([DK, PKW], FP16)       # pos_k^T window
            qct_sb = rpool.tile([DK, NQ], FP16)        # (Q/8 + rcb)^T
            qpt_sb = rpool.tile([DK, NQ], FP16)        # (Q/8 + rpb)^T

            nc.vector.memset(vp_sb[:, :, DV:DV + 1], 1.0)

            with (
                tc.tile_pool(name="ub", bufs=2) as upool,
                tc.tile_pool(name="usk", bufs=2) as uskpool,
                tc.tile_pool(name="ec", bufs=4) as ecpool,
                tc.tile_pool(name="at", bufs=2) as apool,
                tc.tile_pool(name="att", bufs=2) as atpool,
                tc.tile_pool(name="zz", bufs=4) as zpool,
                tc.tile_pool(name="ot", bufs=2) as opool,
                tc.tile_pool(name="fin", bufs=2) as fpool,
                tc.tile_pool(name="psU", bufs=2, space="PSUM") as psU,
                tc.tile_pool(name="psC", bufs=3, space="PSUM") as psC,
                tc.tile_pool(name="psT", bufs=1, space="PSUM") as psT,
                tc.tile_pool(name="psO", bufs=1, space="PSUM") as psO,
                tc.tile_pool(name="psP", bufs=1, space="PSUM") as psP,
            ):
                # ---------- phase A: projections ----------
                # pos_k^T window (only needs post + wpos)
                for mc in range(12):
                    pk_ps = psU.tile([128, 512], F32, tag="psu")
                    nc.tensor.matmul(pk_ps[0:DK, :], wpos_sb[:],
                                     post_sb[:, mc * 512:(mc + 1) * 512],
                                     start=True, stop=True)
                    nc.vector.tensor_copy(
                        pkt_sb[:, mc * 512:(mc + 1) * 512], pk_ps[0:DK, :])
                # Q (+ biases, /8) -> qct, qpt   (only needs xq + wq)
                for qc in range(4):
                    q_ps = psC.tile([128, 512], F32, tag="psc")
                    for cc in range(6):
                        nc.tensor.matmul(q_ps[0:DK, :], wq_sb[:, cc, :],
                                         xq_sb[:, cc, qc * 512:(qc + 1) * 512],
                                         start=(cc == 0), stop=(cc == 5))
                    nc.scalar.activation(qct_sb[:, qc * 512:(qc + 1) * 512],
                                         q_ps[0:DK, :], ACT.Identity,
                                         bias=rcb_sb[:], scale=0.125)
                    nc.scalar.activation(qpt_sb[:, qc * 512:(qc + 1) * 512],
                                         q_ps[0:DK, :], ACT.Identity,
                                         bias=rpb_sb[:], scale=0.125)

                def emit_u_chunk(qt, uc, u_sb):
                    """one chunk of u_sb = exp(U); U[p,m] = y_p . pkt[ws+m]."""
                    ws = 1920 - 128 * qt
                    w = 512 if uc < 8 else UW - 8 * 512
                    u_ps = psU.tile([128, 512], F32, tag="psu")
                    nc.tensor.matmul(
                        u_ps[:, 0:w],
                        qpt_sb[:, qt * 128:(qt + 1) * 128],
                        pkt_sb[:, ws + uc * 512: ws + uc * 512 + w],
                        start=True, stop=True)
                    nc.scalar.activation(
                        u_sb[:, uc * 512: uc * 512 + w], u_ps[:, 0:w],
                        ACT.Exp)

                def emit_u_dma(u_sb):
                    u_dr = dpool.tile([128, UP], BF16, tag="udr")
                    nc.gpsimd.dma_start(u_dr[:], u_sb[:])
                    return u_dr

                def emit_u(qt):
                    u_sb = upool.tile([128, UP], BF16, tag="u")
                    for uc in range(9):
                        emit_u_chunk(qt, uc, u_sb)
                    return emit_u_dma(u_sb)

                def emit_skew(u_dr):
                    """eusk[p, j] = expU[p, j + 127 - p] via strided read."""
                    usk = uskpool.tile([128, L], BF16, tag="usk")
                    skew = bass.AP(u_dr.tensor, u_dr[:].offset + 127,
                                   [[UW, 128], [1, L]])
                    nc.gpsimd.dma_start(usk[:], skew)
                    return usk

                def emit_content(qt, usk, chunks):
                    """content matmul + exp + multiply-by-exp(shifted pos)."""
                    attn = state[qt]["attn"]
                    for ch in chunks:
                        c_ps = psC.tile([128, 512], F32, tag="psc")
                        nc.tensor.matmul(
                            c_ps[:], qct_sb[:, qt * 128:(qt + 1) * 128],
                            kt_sb[:, ch * 512:(ch + 1) * 512],
                            start=True, stop=True)
                        ec = ecpool.tile([128, 512], BF16, tag="ec")
                        nc.scalar.activation(ec[:], c_ps[:], ACT.Exp)
                        nc.vector.tensor_tensor(
                            attn[:, ch * 512:(ch + 1) * 512], ec[:],
                            usk[:, ch * 512:(ch + 1) * 512], ALU.mult)

                def emit_transpose(qt, half):
                    """attnT[j, kb, q] = attn[q, 128*kb + j] via DMA XBAR.

                    Issued per 2048-key half so the transpose overlaps the
                    exp/multiply of the remaining chunks.
                    """
                    attn = state[qt]["attn"]
                    if half == 0:
                        attnT = atpool.tile([128, 32, 128], BF16, tag="attnT")
                        state[qt]["attnT"] = attnT
                    attnT = state[qt]["attnT"]
                    nc.sync.dma_start_transpose(
                        attnT[:, half * 16:(half + 1) * 16, :],
                        attn[:, half * 2048:(half + 1) * 2048])

                def emit_oacc_proj(qt):
                    attnT = state[qt]["attnT"]
                    o_ps = psO.tile([128, DV + 1], F32, tag="po")
                    for kb in range(32):
                        nc.tensor.matmul(o_ps[:], attnT[:, kb, :],
                                         vp_sb[:, kb, :],
                                         start=(kb == 0), stop=(kb == 31))
                    rz = zpool.tile([128, 1], F32, tag="rz")
                    nc.vector.reciprocal(rz[:], o_ps[:, DV:DV + 1])
                    o_sb = opool.tile([128, DV], BF16, tag="o")
                    nc.vector.tensor_copy(o_sb[:], o_ps[:, 0:DV])
                    # transpose o -> oT (128+64 partitions)
                    t_ps = psT.tile([128, 512], BF16, tag="pst")
                    nc.tensor.transpose(t_ps[:, 0:128], o_sb[:, 0:128],
                                        ident_sb[:])
                    nc.tensor.transpose(t_ps[0:64, 128:256], o_sb[:, 128:192],
                                        ident_sb[:])
                    oT1 = opool.tile([128, 128], BF16, tag="oT1")
                    nc.vector.tensor_copy(oT1[:], t_ps[:, 0:128])
                    oT2 = opool.tile([64, 128], BF16, tag="oT2")
                    nc.vector.tensor_copy(oT2[:], t_ps[0:64, 128:256])
                    fin = fpool.tile([128, C], FP16, tag="fin")
                    for n0 in (0, 384):
                        p_ps = psP.tile([128, 384], F32, tag="pp")
                        nc.tensor.matmul(p_ps[:], oT1[:],
                                         wout1_sb[:, n0:n0 + 384],
                                         start=True, stop=False)
                        nc.tensor.matmul(p_ps[:], oT2[:],
                                         wout2_sb[:, n0:n0 + 384],
                                         start=False, stop=True)
                        nc.vector.tensor_scalar_mul(fin[:, n0:n0 + 384],
                                                    p_ps[:], rz[:])
                    nc.gpsimd.dma_start(
                        out_dram[qt * 128:(qt + 1) * 128, :], fin[:])

                state = [dict() for _ in range(NT)]
                # prime U(0) early: it only needs qpt + pkt (fills the
                # tensor-engine gap while xt still streams in for K)
                state[0]["udr"] = emit_u(0)

                # K^T (needs xt)
                for kc in range(8):
                    kt_ps = psC.tile([128, 512], F32, tag="psc")
                    for cc in range(6):
                        nc.tensor.matmul(kt_ps[0:DK, :], wk_sb[:, cc, :],
                                         xt_sb[:, cc, kc * 512:(kc + 1) * 512],
                                         start=(cc == 0), stop=(cc == 5))
                    if kc % 2 == 0:
                        nc.vector.tensor_copy(
                            kt_sb[:, kc * 512:(kc + 1) * 512], kt_ps[0:DK, :])
                    else:
                        nc.scalar.copy(
                            kt_sb[:, kc * 512:(kc + 1) * 512], kt_ps[0:DK, :])

                state[1]["udr"] = emit_u(1)
                state[0]["usk"] = emit_skew(state[0]["udr"])

                # V (key-major, ones column preset)
                for jb in range(32):
                    v_ps = psC.tile([128, 512], F32, tag="psc")
                    for cc in range(6):
                        nc.tensor.matmul(
                            v_ps[:, 0:DV],
                            xt_sb[:, cc, jb * 128:(jb + 1) * 128],
                            wv_sb[:, cc, :], start=(cc == 0), stop=(cc == 5))
                    nc.vector.tensor_copy(vp_sb[:, jb, 0:DV], v_ps[:, 0:DV])

                # ---------- phase B steady loop ----------
                # Per iteration t: content/exp/mult for tile t interleaved
                # with the U matmuls for tile t+2 (hides the per-chunk PSUM
                # drain waits of both streams), XBAR transposes per half,
                # o-acc + projection for tile t-1 as tensor-engine filler.
                for t in range(NT):
                    attn_t = apool.tile([128, L], BF16, tag="attn")
                    state[t]["attn"] = attn_t
                    usk_t = state[t]["usk"]
                    u_next = None
                    if t + 2 < NT:
                        u_next = upool.tile([128, UP], BF16, tag="u")
                    for i in range(4):
                        emit_content(t, usk_t, [i])
                        if u_next is not None:
                            emit_u_chunk(t + 2, i, u_next)
                    emit_transpose(t, 0)
                    for i in range(4, 7):
                        if u_next is not None:
                            emit_u_chunk(t + 2, i, u_next)
                        emit_content(t, usk_t, [i])
                    if u_next is not None:
                        emit_u_chunk(t + 2, 7, u_next)
                        emit_u_chunk(t + 2, 8, u_next)
                        state[t + 2]["udr"] = emit_u_dma(u_next)
                    if t + 1 < NT:
                        state[t + 1]["usk"] = emit_skew(state[t + 1]["udr"])
                    if t > 0:
                        emit_oacc_proj(t - 1)
                    emit_content(t, usk_t, [7])
                    emit_transpose(t, 1)
                emit_oacc_proj(NT - 1)

    nc.finalize()
    return nc


def _positions_T():
    feat = POSF // 2
    pow_rate = np.exp(np.log(L + 1) / feat).astype(np.float64)
    pos = np.arange(-L + 1, L, dtype=np.float64)                 # (8191,)
    cw = pow_rate ** np.arange(1, feat + 1, dtype=np.float64) - 1.0
    emb = (cw[None, :] > np.abs(pos)[:, None]).astype(np.float32)
    signed = np.sign(pos)[:, None].astype(np.float32) * emb
    p = np.concatenate([emb, signed], axis=-1)                   # (8191, 64)
    pt = np.zeros((POSF, 2 * L), np.float32)
    pt[:, :2 * L - 1] = p.T
    return pt


def kernel(x, Wq, Wk, Wv, Wpos, Wout, bout, rel_content_bias, rel_pos_bias):
    bf = ml_dtypes.bfloat16
    f16 = np.float16
    if "nc" not in _nc_cache:
        _nc_cache["nc"] = _build_nc()
    nc = _nc_cache["nc"]

    xt = np.ascontiguousarray(x[0].T).astype(f16)                 # (C, L)
    posT = _positions_T()                                        # (64, 8192)
    ident = np.eye(128, dtype=bf)

    in_maps = []
    for c in range(8):
        h, b = c // 2, c % 2
        w0 = 3968 - 2048 * b
        in_maps.append({
            "xt": xt,
            "xq": np.ascontiguousarray(x[0, b * NQ:(b + 1) * NQ].T).astype(f16),
            "wq": Wq[:, h * DK:(h + 1) * DK].astype(f16),
            "wk": Wk[:, h * DK:(h + 1) * DK].astype(f16),
            "wv": Wv[:, h * DV:(h + 1) * DV].astype(f16),
            "wpos": Wpos[:, h * DK:(h + 1) * DK].astype(f16),
            "post": np.ascontiguousarray(
                posT[:, w0 - 1920: w0 - 1920 + PKW]).astype(f16),
            "wout": Wout[h * DV:(h + 1) * DV, :].astype(bf),
            "rcb": np.ascontiguousarray(
                rel_content_bias[0, h, 0][:, None]).astype(np.float32),
            "rpb": np.ascontiguousarray(
                rel_pos_bias[0, h, 0][:, None]).astype(np.float32),
            "ident": ident,
        })

    res = run_bass_kernel_spmd(nc, in_maps, core_ids=list(range(8)))
    globals()["last_results"] = res
    parts = [r["out"] for r in res.results]

    out = np.zeros((L, C), np.float32)
    for b in range(2):
        acc = np.zeros((NQ, C), np.float32)
        for h in range(4):
            acc += parts[h * 2 + b].astype(np.float32)
        out[b * NQ:(b + 1) * NQ] = acc
    out += bout[None, :].astype(np.float32)
    return out.reshape(1, L, C)
